# revision 7
# baseline (speedup 1.0000x reference)
# GraphSAGE (aggregator='gcn') 2-layer forward on 8 Trainium2 NeuronCores.
#
# Math (per layer): out = D^-1 (A+I) h W + b, D = diag(deg+1).
# Since D^-1 is a row scaling and W a right matmul, we push the matmul
# through the aggregation:  out = D^-1 ((A+I) (h W)) + b.
# This makes layer-2 aggregate 64-wide messages instead of 128-wide.
#
# Sharding: nodes (and their dst-edges) are split into 8 contiguous
# row-slices of 12544 (=98 tiles of 128). Each core:
#   A) computes z1 = x @ W1 for its slice (PE), AllGather -> z1_full
#   C) per dst-tile: dma_gather z1_full[src] (SWDGE, int16-chunked),
#      one-hot via broadcast is_equal (DVE), PSUM-accumulated matmul
#      (PE) => aggT tile; scale by 1/(deg+1) (DVE), +b1,ReLU (ACT);
#      fused z2 tile = h1T.T @ W2 (PE) -> z2_slice; AllGather -> z2_full
#   F) same aggregation over z2_full (64-wide) => output tile.
# Host does only index prep (sort edges by (tile, src-chunk), pad to
# 128-blocks) and the final transpose/concat.

import os
import sys
import types
import contextlib
import ctypes

import numpy as np

N = 100000
E = 1600000
IN, HID, OUT = 128, 128, 64
NC = 8
TILES = 98
SLOTS = TILES * 128          # 12544 rows per core
NSLOT = NC * SLOTS           # 100352 padded rows
CHUNK = 1 << 15              # int16 gather index range


def _install_axon_ntff_hook():
    """run_bass_kernel_spmd(trace=True) under axon imports
    antenv.axon_hooks, which this image lacks. Provide it."""
    if 'antenv.axon_hooks' in sys.modules:
        return
    mod = types.ModuleType('antenv.axon_hooks')
    mod._hook = None
    mod.set_axon_ntff_profile_hook = lambda h: setattr(mod, '_hook', h)
    mod.get_axon_ntff_profile_hook = lambda: mod._hook
    sys.modules['antenv.axon_hooks'] = mod
    try:
        import antenv
        antenv.axon_hooks = mod
        from trn_agent_boot.trn_boot import _ntff_profile_via_ctypes
        mod.set_axon_ntff_profile_hook(
            _ntff_profile_via_ctypes('/opt/axon/libaxon_pjrt.so'))
    except Exception:
        pass


def _prep_host(x, src, dst):
    """Per-core gather-index / one-hot-index tables.

    Returns (B, Boff, TB, per_core dict arrays)."""
    src = np.asarray(src).astype(np.int64).ravel()
    dst = np.asarray(dst).astype(np.int64).ravel()

    deg = np.bincount(dst, minlength=N)
    invdegp1 = (1.0 / (deg + 1.0)).astype(np.float32)

    # edges + self loops
    s_all = np.concatenate([src, np.arange(N, dtype=np.int64)])
    d_all = np.concatenate([dst, np.arange(N, dtype=np.int64)])

    chunk = s_all >> 15                      # 0..3
    gtile = d_all >> 7                       # global tile 0..783
    nchunks = (NSLOT + CHUNK - 1) // CHUNK   # 4
    key = gtile * nchunks + chunk
    order = np.argsort(key, kind='stable')
    ks = key[order]
    ss = s_all[order]
    ds = d_all[order]

    counts = np.bincount(key, minlength=NC * TILES * nchunks)
    # per-(tile, chunk) block count: max over cores so the IR is SPMD
    cpt = counts.reshape(NC, TILES, nchunks)
    Btc = np.ceil(cpt.max(axis=0) / 128.0).astype(np.int64)  # [TILES, nchunks]
    # per-tile chunk block offsets and per-tile totals
    Boff_t = np.zeros((TILES, nchunks + 1), np.int64)
    Boff_t[:, 1:] = np.cumsum(Btc, axis=1)
    TB_t = Boff_t[:, -1]                       # blocks per tile
    tile_off = np.zeros(TILES + 1, np.int64)   # block offset of tile start
    tile_off[1:] = np.cumsum(TB_t)
    TOTB = int(tile_off[-1])

    starts = np.zeros(NC * TILES * nchunks + 1, np.int64)
    starts[1:] = np.cumsum(counts)
    within = np.arange(len(ks), dtype=np.int64) - starts[ks]

    core = ds // SLOTS
    t = (ds % SLOTS) >> 7
    chs = ks % nchunks
    blk = tile_off[t] + Boff_t[t, chs]         # block index of region start

    # gather indices, wrapped (slot i -> row i%16, col i//16), 16-row
    # pattern replicated to 128 partitions
    idx_cols = TOTB * 8
    idx_arr = np.zeros((NC, 16, idx_cols), np.int16)
    wcol = blk * 8 + (within >> 4)
    idx_arr[core, within % 16, wcol] = (ss & (CHUNK - 1)).astype(np.int16)
    idx_arr = np.tile(idx_arr, (1, 8, 1))

    # local-dst table (255 = padding slot, killed by the one-hot)
    ldst_arr = np.full((NC, 128, TOTB), 255.0, np.float32)
    bcol = blk + (within >> 7)
    ldst_arr[core, within % 128, bcol] = (ds & 127).astype(np.float32)

    xT = np.zeros((128, NSLOT), np.float32)
    xT[:, :N] = np.asarray(x, dtype=np.float32).T

    ivd = np.ones(NSLOT, np.float32)
    ivd[:N] = invdegp1

    per_core = []
    for c in range(NC):
        sl = slice(c * SLOTS, (c + 1) * SLOTS)
        per_core.append({
            "xT": np.ascontiguousarray(xT[:, sl]),
            "idx": np.ascontiguousarray(idx_arr[c]),
            "ldst": np.ascontiguousarray(ldst_arr[c]),
            "invd": np.ascontiguousarray(
                np.broadcast_to(ivd[sl][None, :], (128, SLOTS))),
        })
    return Btc, Boff_t, tile_off, TOTB, per_core


def _build_program(Btc, Boff_t, tile_off, TOTB):
    import concourse.bacc as bacc
    import concourse.bass as bass
    import concourse.mybir as mybir
    import concourse.tile as tile

    f32 = mybir.dt.float32
    f32r = mybir.dt.float32r
    i16 = mybir.dt.int16
    AF = mybir.ActivationFunctionType
    nchunks = Btc.shape[1]
    TBmax = int((Boff_t[:, -1]).max())
    use_f32r = os.environ.get("KERNEL_FP32R", "1") == "1"
    mmdt = f32r if use_f32r else f32

    def mm_cast(ap):
        # bitcast a float32 AP for storage into / reading from an f32r tile
        return ap.bitcast(f32r) if use_f32r else ap

    nc = bacc.Bacc("TRN2", target_bir_lowering=False, debug=False,
                   num_swdge_queues=4)

    xT = nc.dram_tensor("xT", [128, SLOTS], f32, kind="ExternalInput")
    W1 = nc.dram_tensor("W1", [IN, HID], f32, kind="ExternalInput")
    b1 = nc.dram_tensor("b1", [HID, 1], f32, kind="ExternalInput")
    W2 = nc.dram_tensor("W2", [HID, OUT], f32, kind="ExternalInput")
    b2 = nc.dram_tensor("b2", [OUT, 1], f32, kind="ExternalInput")
    iota = nc.dram_tensor("iota", [128, 128], f32, kind="ExternalInput")
    idxT = nc.dram_tensor("idx", [128, TOTB * 8], i16,
                          kind="ExternalInput")
    ldstT = nc.dram_tensor("ldst", [128, TOTB], f32,
                           kind="ExternalInput")
    invd = nc.dram_tensor("invd", [128, SLOTS], f32, kind="ExternalInput")
    outT = nc.dram_tensor("outT", [OUT, SLOTS], f32, kind="ExternalOutput")
    z1_full = nc.dram_tensor("z1_full", [NSLOT, HID], f32,
                             addr_space="Shared")
    z2_full = nc.dram_tensor("z2_full", [NSLOT, OUT], f32,
                             addr_space="Shared")

    rg = [list(range(NC))]
    ts = bass.ts

    with tile.TileContext(nc) as tc:
        with tc.tile_pool(name="const", bufs=1) as cp, \
             tc.tile_pool(name="sb", bufs=3) as sb, \
             tc.tile_pool(name="deep", bufs=4) as dq, \
             tc.tile_pool(name="ps", bufs=2, space="PSUM") as ps, \
             tc.tile_pool(name="dram", bufs=1, space="DRAM") as dp:

            W1s = cp.tile([IN, HID], mmdt, tag="W1")
            nc.sync.dma_start(out=W1s[:], in_=mm_cast(W1[:]))
            W2s = cp.tile([HID, OUT], mmdt, tag="W2")
            nc.sync.dma_start(out=W2s[:], in_=mm_cast(W2[:]))
            b1s = cp.tile([HID, 1], f32, tag="b1")
            nc.sync.dma_start(out=b1s[:], in_=b1[:])
            b2s = cp.tile([OUT, 1], f32, tag="b2")
            nc.sync.dma_start(out=b2s[:], in_=b2[:])
            iotas = cp.tile([128, 128], f32, tag="iota")
            nc.sync.dma_start(out=iotas[:], in_=iota[:])
            ldsts = cp.tile([128, TOTB], f32, tag="ldst")
            nc.sync.dma_start(out=ldsts[:], in_=ldstT[:])

            z1_slice = dp.tile([SLOTS, HID], f32, tag="z1s")
            z2_slice = dp.tile([SLOTS, OUT], f32, tag="z2s")

            # Phase A: z1 slice = x @ W1
            for t in range(TILES):
                xt = sb.tile([128, 128], mmdt, tag="xt")
                nc.sync.dma_start(out=xt[:], in_=mm_cast(xT[:, ts(t, 128)]))
                pa = ps.tile([128, 128], f32, tag="mmA")
                nc.tensor.matmul(out=pa[:], lhsT=xt[:], rhs=W1s[:],
                                 start=True, stop=True)
                zt = sb.tile([128, 128], f32, tag="zt")
                nc.vector.tensor_copy(out=zt[:], in_=pa[:])
                nc.sync.dma_start(out=z1_slice[ts(t, 128), :], in_=zt[:])

            nc.gpsimd.collective_compute(
                "AllGather", mybir.AluOpType.bypass, replica_groups=rg,
                ins=[z1_slice.opt()], outs=[z1_full[:]])

            # Phase C: layer-1 aggregation + fused z2 compute
            qrr = [0]
            for t in range(TILES):
                TB_t = int(Boff_t[t, -1])
                t0 = int(tile_off[t])
                idxs = dq.tile([128, TBmax * 8], i16, tag="idxs")
                nc.sync.dma_start(out=idxs[:, :TB_t * 8],
                                  in_=idxT[:, t0 * 8:(t0 + TB_t) * 8])
                msg = dq.tile([128, TBmax, 128], mmdt, tag="msg1")
                for k in range(nchunks):
                    Bk = int(Btc[t, k])
                    if Bk == 0:
                        continue
                    lo = k * CHUNK
                    hi = min((k + 1) * CHUNK, NSLOT)
                    bo = int(Boff_t[t, k])
                    nc.gpsimd.dma_gather(
                        out_ap=msg[:, bo:bo + Bk, :],
                        in_ap=mm_cast(z1_full[lo:hi, :]),
                        idxs_ap=idxs[:, bo * 8:(bo + Bk) * 8],
                        num_idxs=Bk * 128, num_idxs_reg=Bk * 128,
                        elem_size=HID, queue_num=qrr[0] % 4)
                    qrr[0] += 1
                oh = dq.tile([128, TBmax, 128], mmdt, tag="oh")
                nc.vector.tensor_tensor(
                    out=oh[:, :TB_t, :],
                    in0=ldsts[:, t0:t0 + TB_t].unsqueeze(2)
                        .broadcast_to([128, TB_t, 128]),
                    in1=iotas[:].unsqueeze(1).broadcast_to([128, TB_t, 128]),
                    op=mybir.AluOpType.is_equal)
                p1 = ps.tile([128, 128], f32, tag="mmA")
                for b in range(TB_t):
                    nc.tensor.matmul(out=p1[:], lhsT=msg[:, b, :],
                                     rhs=oh[:, b, :],
                                     start=(b == 0), stop=(b == TB_t - 1))
                ivt = sb.tile([128, 128], f32, tag="ivt")
                nc.sync.dma_start(out=ivt[:], in_=invd[:, ts(t, 128)])
                h1m = sb.tile([128, 128], f32, tag="h1m")
                nc.vector.tensor_mul(out=h1m[:], in0=p1[:], in1=ivt[:])
                h1 = sb.tile([128, 128], mmdt, tag="h1")
                nc.scalar.activation(out=h1[:], in_=h1m[:], func=AF.Relu,
                                     bias=b1s[:], scale=1.0)
                pz = ps.tile([128, OUT], f32, tag="mmS")
                nc.tensor.matmul(out=pz[:], lhsT=h1[:], rhs=W2s[:],
                                 start=True, stop=True)
                z2t = sb.tile([128, OUT], f32, tag="z2t")
                nc.vector.tensor_copy(out=z2t[:], in_=pz[:])
                nc.sync.dma_start(out=z2_slice[ts(t, 128), :], in_=z2t[:])

            nc.gpsimd.collective_compute(
                "AllGather", mybir.AluOpType.bypass, replica_groups=rg,
                ins=[z2_slice.opt()], outs=[z2_full[:]])

            # Phase F: layer-2 aggregation
            for t in range(TILES):
                TB_t = int(Boff_t[t, -1])
                t0 = int(tile_off[t])
                idxs = dq.tile([128, TBmax * 8], i16, tag="idxs")
                nc.sync.dma_start(out=idxs[:, :TB_t * 8],
                                  in_=idxT[:, t0 * 8:(t0 + TB_t) * 8])
                m2 = dq.tile([128, TBmax, OUT], mmdt, tag="msg2")
                for k in range(nchunks):
                    Bk = int(Btc[t, k])
                    if Bk == 0:
                        continue
                    lo = k * CHUNK
                    hi = min((k + 1) * CHUNK, NSLOT)
                    bo = int(Boff_t[t, k])
                    nc.gpsimd.dma_gather(
                        out_ap=m2[:, bo:bo + Bk, :],
                        in_ap=mm_cast(z2_full[lo:hi, :]),
                        idxs_ap=idxs[:, bo * 8:(bo + Bk) * 8],
                        num_idxs=Bk * 128, num_idxs_reg=Bk * 128,
                        elem_size=OUT, queue_num=qrr[0] % 4)
                    qrr[0] += 1
                oh = dq.tile([128, TBmax, 128], mmdt, tag="oh")
                nc.vector.tensor_tensor(
                    out=oh[:, :TB_t, :],
                    in0=ldsts[:, t0:t0 + TB_t].unsqueeze(2)
                        .broadcast_to([128, TB_t, 128]),
                    in1=iotas[:].unsqueeze(1).broadcast_to([128, TB_t, 128]),
                    op=mybir.AluOpType.is_equal)
                p2 = ps.tile([OUT, 128], f32, tag="mmS")
                for b in range(TB_t):
                    nc.tensor.matmul(out=p2[:], lhsT=m2[:, b, :],
                                     rhs=oh[:, b, :],
                                     start=(b == 0), stop=(b == TB_t - 1))
                ivt = sb.tile([128, 128], f32, tag="ivt")
                nc.sync.dma_start(out=ivt[:], in_=invd[:, ts(t, 128)])
                ot = sb.tile([OUT, 128], f32, tag="ot")
                nc.vector.tensor_mul(out=ot[:], in0=p2[:], in1=ivt[:OUT, :])
                nc.scalar.activation(out=ot[:], in_=ot[:], func=AF.Identity,
                                     bias=b2s[:], scale=1.0)
                nc.sync.dma_start(out=outT[:, ts(t, 128)], in_=ot[:])

    nc.compile()
    return nc


LAST_RESULTS = None


def kernel(x, src, dst, W1, b1, W2, b2):
    global LAST_RESULTS
    _install_axon_ntff_hook()
    from concourse.bass_utils import run_bass_kernel_spmd

    Btc, Boff_t, tile_off, TOTB, per_core = _prep_host(x, src, dst)
    nc = _build_program(Btc, Boff_t, tile_off, TOTB)

    W1f = np.ascontiguousarray(np.asarray(W1, dtype=np.float32))
    W2f = np.ascontiguousarray(np.asarray(W2, dtype=np.float32))
    b1f = np.ascontiguousarray(
        np.asarray(b1, dtype=np.float32).reshape(HID, 1))
    b2f = np.ascontiguousarray(
        np.asarray(b2, dtype=np.float32).reshape(OUT, 1))
    iota = np.ascontiguousarray(
        np.tile(np.arange(128, dtype=np.float32)[None, :], (128, 1)))

    in_maps = []
    for c in range(NC):
        pc = per_core[c]
        in_maps.append({
            "xT": pc["xT"], "idx": pc["idx"], "ldst": pc["ldst"],
            "invd": pc["invd"], "W1": W1f, "b1": b1f, "W2": W2f,
            "b2": b2f, "iota": iota,
        })

    res = run_bass_kernel_spmd(nc, in_maps, core_ids=list(range(NC)))
    LAST_RESULTS = res

    out = np.concatenate(
        [res.results[c]["outT"].T for c in range(NC)], axis=0)
    return np.ascontiguousarray(out[:N])


# revision 9
# speedup vs baseline: 1.1110x; 1.1110x over previous
# GraphSAGE (aggregator='gcn') 2-layer forward on 8 Trainium2 NeuronCores.
#
# Math (per layer): out = D^-1 (A+I) h W + b, D = diag(deg+1).
# Since D^-1 is a row scaling and W a right matmul, we push the matmul
# through the aggregation:  out = D^-1 ((A+I) (h W)) + b.
#
# Sharding: nodes (and their dst-edges) are split into 8 contiguous
# row-slices of 12544 (=98 tiles of 128). Each core:
#   A) z1 = x @ W1 for its slice (PE), AllGather -> z1_full (bf16)
#   C) per dst-tile: dma_gather z1_full[src] (SWDGE ucode, int16 chunked
#      indices, 4 queues), one-hot via broadcast is_equal (DVE),
#      PSUM-accumulated bf16 matmuls (PE) => aggT tile; scale by
#      1/(deg+1) (DVE, resident table), +b1+ReLU (ACT); fused
#      z2 tile = h1T.T @ W2 (PE) -> z2_slice; AllGather -> z2_full
#   F) same aggregation over z2_full (row-padded bf16) => output tile.
# Host does only index prep (bucket edges by (tile, src-chunk), pad to
# 128-blocks) and the final transpose/concat.

import os
import sys
import types

import numpy as np

N = 100000
E = 1600000
IN, HID, OUT = 128, 128, 64
NC = 8
TILES = 98
SLOTS = TILES * 128          # 12544 rows per core
NSLOT = NC * SLOTS           # 100352 padded rows
CHUNK = 1 << 15              # int16 gather index range


def _install_axon_ntff_hook():
    """run_bass_kernel_spmd(trace=True) under axon imports
    antenv.axon_hooks, which this image lacks. Provide it."""
    if 'antenv.axon_hooks' in sys.modules:
        return
    mod = types.ModuleType('antenv.axon_hooks')
    mod._hook = None
    mod.set_axon_ntff_profile_hook = lambda h: setattr(mod, '_hook', h)
    mod.get_axon_ntff_profile_hook = lambda: mod._hook
    sys.modules['antenv.axon_hooks'] = mod
    try:
        import antenv
        antenv.axon_hooks = mod
        from trn_agent_boot.trn_boot import _ntff_profile_via_ctypes
        mod.set_axon_ntff_profile_hook(
            _ntff_profile_via_ctypes('/opt/axon/libaxon_pjrt.so'))
    except Exception:
        pass


def _prep_host(x, src, dst):
    """Per-core gather-index / one-hot-index tables."""
    src = np.asarray(src).astype(np.int64).ravel()
    dst = np.asarray(dst).astype(np.int64).ravel()

    deg = np.bincount(dst, minlength=N)
    invdegp1 = (1.0 / (deg + 1.0)).astype(np.float32)

    # edges + self loops
    s_all = np.concatenate([src, np.arange(N, dtype=np.int64)])
    d_all = np.concatenate([dst, np.arange(N, dtype=np.int64)])

    chunk = s_all >> 15                      # 0..3
    gtile = d_all >> 7                       # global tile 0..783
    nchunks = (NSLOT + CHUNK - 1) // CHUNK   # 4
    key = gtile * nchunks + chunk
    order = np.argsort(key, kind='stable')
    ks = key[order]
    ss = s_all[order]
    ds = d_all[order]

    counts = np.bincount(key, minlength=NC * TILES * nchunks)
    # per-(tile, chunk) block count: max over cores so the IR is SPMD
    cpt = counts.reshape(NC, TILES, nchunks)
    Btc = np.ceil(cpt.max(axis=0) / 128.0).astype(np.int64)  # [TILES, nch]
    Boff_t = np.zeros((TILES, nchunks + 1), np.int64)
    Boff_t[:, 1:] = np.cumsum(Btc, axis=1)
    tile_off = np.zeros(TILES + 1, np.int64)
    tile_off[1:] = np.cumsum(Boff_t[:, -1])
    TOTB = int(tile_off[-1])

    starts = np.zeros(NC * TILES * nchunks + 1, np.int64)
    starts[1:] = np.cumsum(counts)
    within = np.arange(len(ks), dtype=np.int64) - starts[ks]

    core = ds // SLOTS
    t = (ds % SLOTS) >> 7
    chs = ks % nchunks
    blk = tile_off[t] + Boff_t[t, chs]

    # gather indices, wrapped (slot i -> row i%16, col i//16), 16-row
    # pattern replicated to 128 partitions
    idx_arr = np.zeros((NC, 16, TOTB * 8), np.int16)
    wcol = blk * 8 + (within >> 4)
    idx_arr[core, within % 16, wcol] = (ss & (CHUNK - 1)).astype(np.int16)
    idx_arr = np.tile(idx_arr, (1, 8, 1))

    # local-dst table (255 = padding slot, killed by the one-hot)
    ldst_arr = np.full((NC, 128, TOTB), 255.0, np.float32)
    bcol = blk + (within >> 7)
    ldst_arr[core, within % 128, bcol] = (ds & 127).astype(np.float32)

    xT = np.zeros((128, NSLOT), np.float32)
    xT[:, :N] = np.asarray(x, dtype=np.float32).T

    ivd = np.ones(NSLOT, np.float32)
    ivd[:N] = invdegp1

    per_core = []
    for c in range(NC):
        sl = slice(c * SLOTS, (c + 1) * SLOTS)
        per_core.append({
            "xT": np.ascontiguousarray(xT[:, sl]),
            "idx": np.ascontiguousarray(idx_arr[c]),
            "ldst": np.ascontiguousarray(ldst_arr[c]),
            "invd": np.ascontiguousarray(
                np.broadcast_to(ivd[sl][None, :], (128, SLOTS))),
        })
    return Btc, Boff_t, tile_off, TOTB, per_core


def _build_program(Btc, Boff_t, tile_off, TOTB):
    import concourse.bacc as bacc
    import concourse.bass as bass
    import concourse.mybir as mybir
    import concourse.tile as tile

    f32 = mybir.dt.float32
    f32r = mybir.dt.float32r
    bf16 = mybir.dt.bfloat16
    i16 = mybir.dt.int16
    AF = mybir.ActivationFunctionType
    nchunks = Btc.shape[1]
    TBmax = int((Boff_t[:, -1]).max())
    Bmax_k = [int(Btc[:, k].max()) for k in range(nchunks)]

    nc = bacc.Bacc("TRN2", target_bir_lowering=False, debug=False,
                   num_swdge_queues=4)

    xT = nc.dram_tensor("xT", [128, SLOTS], f32, kind="ExternalInput")
    W1 = nc.dram_tensor("W1", [IN, HID], f32, kind="ExternalInput")
    b1 = nc.dram_tensor("b1", [HID, 1], f32, kind="ExternalInput")
    W2 = nc.dram_tensor("W2", [HID, OUT], f32, kind="ExternalInput")
    b2 = nc.dram_tensor("b2", [OUT, 1], f32, kind="ExternalInput")
    iota = nc.dram_tensor("iota", [128, 128], f32, kind="ExternalInput")
    idxT = nc.dram_tensor("idx", [128, TOTB * 8], i16, kind="ExternalInput")
    ldstT = nc.dram_tensor("ldst", [128, TOTB], f32, kind="ExternalInput")
    invd = nc.dram_tensor("invd", [128, SLOTS], f32, kind="ExternalInput")
    outT = nc.dram_tensor("outT", [OUT, SLOTS], f32, kind="ExternalOutput")
    # bf16 message tables; z2 rows padded to 128 cols so the gathered
    # element stays a multiple of 256 bytes (cols 64.. are never read)
    z1_full = nc.dram_tensor("z1_full", [NSLOT, HID], bf16,
                             addr_space="Shared")
    z2_full = nc.dram_tensor("z2_full", [NSLOT, 128], bf16,
                             addr_space="Shared")

    rg = [list(range(NC))]
    ts = bass.ts

    with tile.TileContext(nc) as tc:
        with tc.tile_pool(name="const", bufs=1) as cp, \
             tc.tile_pool(name="sb", bufs=3) as sb, \
             tc.tile_pool(name="deep", bufs=6) as dq, \
             tc.tile_pool(name="ps", bufs=2, space="PSUM") as ps, \
             tc.tile_pool(name="dram", bufs=1, space="DRAM") as dp:

            W1s = cp.tile([IN, HID], f32r, tag="W1")
            nc.sync.dma_start(out=W1s[:], in_=W1[:].bitcast(f32r))
            W2s = cp.tile([HID, OUT], f32r, tag="W2")
            nc.sync.dma_start(out=W2s[:], in_=W2[:].bitcast(f32r))
            b1s = cp.tile([HID, 1], f32, tag="b1")
            nc.sync.dma_start(out=b1s[:], in_=b1[:])
            b2s = cp.tile([OUT, 1], f32, tag="b2")
            nc.sync.dma_start(out=b2s[:], in_=b2[:])
            iotas = cp.tile([128, 128], f32, tag="iota")
            nc.sync.dma_start(out=iotas[:], in_=iota[:])
            idxs = cp.tile([128, TOTB * 8], i16, tag="idx")
            nc.sync.dma_start(out=idxs[:], in_=idxT[:])
            ldsts = cp.tile([128, TOTB], f32, tag="ldst")
            nc.sync.dma_start(out=ldsts[:], in_=ldstT[:])
            invds = cp.tile([128, SLOTS], f32, tag="invd")
            nc.sync.dma_start(out=invds[:], in_=invd[:])

            z1_slice = dp.tile([SLOTS, HID], bf16, tag="z1s")
            z2_slice = dp.tile([SLOTS, 128], bf16, tag="z2s")

            # Phase A: z1 slice = x @ W1  (bf16 out)
            for t in range(TILES):
                xt = sb.tile([128, 128], f32r, tag="xt")
                nc.sync.dma_start(out=xt[:],
                                  in_=xT[:, ts(t, 128)].bitcast(f32r))
                pa = ps.tile([128, 128], f32, tag="mmA")
                nc.tensor.matmul(out=pa[:], lhsT=xt[:], rhs=W1s[:],
                                 start=True, stop=True)
                zt = sb.tile([128, 128], bf16, tag="zt")
                nc.vector.tensor_copy(out=zt[:], in_=pa[:])
                nc.sync.dma_start(out=z1_slice[ts(t, 128), :], in_=zt[:])

            nc.gpsimd.collective_compute(
                "AllGather", mybir.AluOpType.bypass, replica_groups=rg,
                ins=[z1_slice.opt()], outs=[z1_full[:]])

            # Phase C: layer-1 aggregation + fused z2 compute
            qrr = [0]
            for t in range(TILES):
                TB_t = int(Boff_t[t, -1])
                t0 = int(tile_off[t])
                msgs = []
                for k in range(nchunks):
                    Bk = int(Btc[t, k])
                    if Bk == 0:
                        msgs.append(None)
                        continue
                    mk = dq.tile([128, int(Bmax_k[k]), 128], bf16,
                                 tag=f"msg{k}")
                    msgs.append(mk)
                    lo = k * CHUNK
                    hi = min((k + 1) * CHUNK, NSLOT)
                    bo = int(Boff_t[t, k])
                    c0 = (t0 + bo) * 8
                    nc.gpsimd.dma_gather(
                        out_ap=mk[:, :Bk, :],
                        in_ap=z1_full[lo:hi, :],
                        idxs_ap=idxs[:, c0:c0 + Bk * 8],
                        num_idxs=Bk * 128, num_idxs_reg=Bk * 128,
                        elem_size=HID, queue_num=qrr[0] % 4)
                    qrr[0] += 1
                oh = dq.tile([128, TBmax, 128], bf16, tag="oh")
                nc.vector.tensor_tensor(
                    out=oh[:, :TB_t, :],
                    in0=ldsts[:, t0:t0 + TB_t].unsqueeze(2)
                        .broadcast_to([128, TB_t, 128]),
                    in1=iotas[:].unsqueeze(1).broadcast_to([128, TB_t, 128]),
                    op=mybir.AluOpType.is_equal)
                p1 = ps.tile([128, 128], f32, tag="mmA")
                gb = 0
                for k in range(nchunks):
                    Bk = int(Btc[t, k])
                    for b in range(Bk):
                        nc.tensor.matmul(out=p1[:], lhsT=msgs[k][:, b, :],
                                         rhs=oh[:, gb, :],
                                         start=(gb == 0),
                                         stop=(gb == TB_t - 1))
                        gb += 1
                h1m = sb.tile([128, 128], f32, tag="h1m")
                nc.vector.tensor_mul(out=h1m[:], in0=p1[:],
                                     in1=invds[:, ts(t, 128)])
                h1 = sb.tile([128, 128], f32r, tag="h1")
                nc.scalar.activation(out=h1[:], in_=h1m[:], func=AF.Relu,
                                     bias=b1s[:], scale=1.0)
                pz = ps.tile([128, OUT], f32, tag="mmS")
                nc.tensor.matmul(out=pz[:], lhsT=h1[:], rhs=W2s[:],
                                 start=True, stop=True)
                z2t = sb.tile([128, OUT], bf16, tag="z2t")
                nc.vector.tensor_copy(out=z2t[:], in_=pz[:])
                nc.sync.dma_start(out=z2_slice[ts(t, 128), :OUT], in_=z2t[:])

            nc.gpsimd.collective_compute(
                "AllGather", mybir.AluOpType.bypass, replica_groups=rg,
                ins=[z2_slice.opt()], outs=[z2_full[:]])

            # Phase F: layer-2 aggregation
            for t in range(TILES):
                TB_t = int(Boff_t[t, -1])
                t0 = int(tile_off[t])
                msgs = []
                for k in range(nchunks):
                    Bk = int(Btc[t, k])
                    if Bk == 0:
                        msgs.append(None)
                        continue
                    mk = dq.tile([128, int(Bmax_k[k]), 128], bf16,
                                 tag=f"msg{k}")
                    msgs.append(mk)
                    lo = k * CHUNK
                    hi = min((k + 1) * CHUNK, NSLOT)
                    bo = int(Boff_t[t, k])
                    c0 = (t0 + bo) * 8
                    nc.gpsimd.dma_gather(
                        out_ap=mk[:, :Bk, :],
                        in_ap=z2_full[lo:hi, :],
                        idxs_ap=idxs[:, c0:c0 + Bk * 8],
                        num_idxs=Bk * 128, num_idxs_reg=Bk * 128,
                        elem_size=128, queue_num=qrr[0] % 4)
                    qrr[0] += 1
                oh = dq.tile([128, TBmax, 128], bf16, tag="oh")
                nc.vector.tensor_tensor(
                    out=oh[:, :TB_t, :],
                    in0=ldsts[:, t0:t0 + TB_t].unsqueeze(2)
                        .broadcast_to([128, TB_t, 128]),
                    in1=iotas[:].unsqueeze(1).broadcast_to([128, TB_t, 128]),
                    op=mybir.AluOpType.is_equal)
                p2 = ps.tile([OUT, 128], f32, tag="mmS")
                gb = 0
                for k in range(nchunks):
                    Bk = int(Btc[t, k])
                    for b in range(Bk):
                        nc.tensor.matmul(out=p2[:], lhsT=msgs[k][:, b, :OUT],
                                         rhs=oh[:, gb, :],
                                         start=(gb == 0),
                                         stop=(gb == TB_t - 1))
                        gb += 1
                ot = sb.tile([OUT, 128], f32, tag="ot")
                nc.vector.tensor_mul(out=ot[:], in0=p2[:],
                                     in1=invds[:OUT, ts(t, 128)])
                nc.scalar.activation(out=ot[:], in_=ot[:], func=AF.Identity,
                                     bias=b2s[:], scale=1.0)
                nc.sync.dma_start(out=outT[:, ts(t, 128)], in_=ot[:])

    nc.compile()
    return nc


LAST_RESULTS = None


def kernel(x, src, dst, W1, b1, W2, b2):
    global LAST_RESULTS
    _install_axon_ntff_hook()
    from concourse.bass_utils import run_bass_kernel_spmd

    Btc, Boff_t, tile_off, TOTB, per_core = _prep_host(x, src, dst)
    nc = _build_program(Btc, Boff_t, tile_off, TOTB)

    W1f = np.ascontiguousarray(np.asarray(W1, dtype=np.float32))
    W2f = np.ascontiguousarray(np.asarray(W2, dtype=np.float32))
    b1f = np.ascontiguousarray(
        np.asarray(b1, dtype=np.float32).reshape(HID, 1))
    b2f = np.ascontiguousarray(
        np.asarray(b2, dtype=np.float32).reshape(OUT, 1))
    iota = np.ascontiguousarray(
        np.tile(np.arange(128, dtype=np.float32)[None, :], (128, 1)))

    in_maps = []
    for c in range(NC):
        pc = per_core[c]
        in_maps.append({
            "xT": pc["xT"], "idx": pc["idx"], "ldst": pc["ldst"],
            "invd": pc["invd"], "W1": W1f, "b1": b1f, "W2": W2f,
            "b2": b2f, "iota": iota,
        })

    res = run_bass_kernel_spmd(nc, in_maps, core_ids=list(range(NC)))
    LAST_RESULTS = res

    out = np.concatenate(
        [res.results[c]["outT"].T for c in range(NC)], axis=0)
    return np.ascontiguousarray(out[:N])
